# revision 1
# baseline (speedup 1.0000x reference)
"""AntiSymmetricConv (GNN message passing) on 8 TRN2 NeuronCores.

Strategy (dst-sharded "pull" mode):
  - Host: compute deg/dinv, sort dst nodes by degree (desc), assign 128-node
    tiles round-robin to 8 cores (load balance + identical static schedule),
    build per-(tile, slot, k) source-index arrays: slot p of a tile holds the
    k-th in-edge of that tile's p-th dst.  Because slot p <-> dst p, the
    scatter-add becomes PSUM accumulation with a *static identity* stationary
    operand - no per-chunk one-hot matrices.
  - Device, per iteration:
      phase A: per node tile: xT = transpose(x_tile) (PE), [xw|xa] = x_tile @
               [W_phi.T | A.T] (one matmul, N=256), y = xw * dinv * valid,
               xa += bias; write y to DRAM bounce.
      AllGather y shards -> y_full [8*NPC, 128] (Shared DRAM).
      phase C: per dst tile: indirect-DMA gather in-edge rows of y_full into
               SBUF [128, D*128], matmul-accumulate (lhsT = identity) in
               groups of 4 chunks (N=512) into one PSUM bank, + self-loop
               matmul from local y; epilogue folds PSUM blocks, h =
               tanh(xa + dinv*agg), x += 0.1*h.
  - Output: read back x shards, invert the permutation on host.
"""

import os

import numpy as np

import concourse.bacc as bacc
import concourse.bass as bass
import concourse.mybir as mybir
import concourse.tile as tile
from concourse.bass_utils import run_bass_kernel_spmd
from concourse.masks import make_identity

GAMMA = 0.1
EPSILON = 0.1
NUM_ITERS = 4
P = 128  # partitions / tile size
C = 8    # cores
D = 128  # feature dim

FP = mybir.dt.float32
I32 = mybir.dt.int32


# ----------------------------------------------------------------- host prep
def _preprocess(x, edge_index, W, W_phi, bias):
    N = x.shape[0]
    E = edge_index.shape[1]
    src, dst = edge_index[0].astype(np.int64), edge_index[1].astype(np.int64)

    deg = np.bincount(dst, minlength=N).astype(np.float64) + 1.0
    dinv = (1.0 / np.sqrt(deg)).astype(np.float32)

    # global degree-descending order of dst nodes
    order = np.argsort(-deg, kind="stable")
    rank = np.empty(N, dtype=np.int64)
    rank[order] = np.arange(N)

    n_tiles_global = -(-N // P)          # 782 for N=100000
    # +1 guarantees at least one all-pad slot (the ZERO row) on core C-1
    NT = -(-(n_tiles_global + 1) // C)   # tiles per core (98)
    NPC = NT * P                         # rows per core (12544)

    # node (by rank r) -> (core, tile_i, slot)
    g = rank // P
    core_of = g % C
    tile_of = g // C
    slot_of = rank % P

    # position of each node's y-row inside y_full ([core][slot][tile] layout,
    # row-major [128, NT, 128] per core => row index = slot*NT + tile)
    pos = core_of * NPC + slot_of * NT + tile_of  # int64 [N]
    ZERO_POS = np.int32((C - 1) * NPC + NPC - 1)  # last row of core 7: all-pad

    # per-edge target placement
    er = rank[dst]
    # order edges by (core, tile, slot) then assign k within each dst
    eorder = np.argsort(er, kind="stable")
    er_s = er[eorder]
    src_s = src[eorder]
    # k-th edge of each dst: running counter within equal er_s
    # (er_s sorted -> use index - first-occurrence)
    first = np.searchsorted(er_s, er_s)
    k_of = np.arange(E) - first

    # per-(core, tile) max in-edge count, then schedule = max over cores
    indeg = (deg - 1.0).astype(np.int64)
    indeg_sorted = indeg[order]  # by rank
    pad_tiles = NT * C - n_tiles_global
    indeg_pad = np.concatenate(
        [indeg_sorted, np.zeros(NT * C * P - N, dtype=np.int64)]
    )
    tile_max = indeg_pad.reshape(NT * C, P).max(axis=1)  # per global tile g
    D_sched = tile_max.reshape(NT, C).max(axis=1)        # per tile position i
    D_sched = np.maximum(D_sched, 1).astype(np.int64)
    CHT = int(D_sched.sum())

    # chunk-column offsets per tile position
    off = np.zeros(NT + 1, dtype=np.int64)
    off[1:] = np.cumsum(D_sched)

    # build src index arrays [C][P, CHT]
    src_arr = np.full((C, P, CHT), ZERO_POS, dtype=np.int32)
    eg = er_s // P                       # global tile of each (sorted) edge
    e_tile = eg // C
    e_core_s = eg % C
    e_slot = er_s % P
    col = off[e_tile] + k_of
    src_arr[e_core_s, e_slot, col] = pos[src_s].astype(np.int32)

    # per-core node data: x_sb[c][p, t*D+f] = x[node at (c, slot p, tile t)]
    node_ids = np.full((C, P, NT), -1, dtype=np.int64)
    node_ids[core_of, slot_of, tile_of] = np.arange(N)
    valid = node_ids >= 0
    nid = np.where(valid, node_ids, 0)
    x_gather = x[nid.reshape(C, -1)]  # [C, P*NT, D]
    x_gather[~valid.reshape(C, -1)] = 0.0
    x_sb = x_gather.reshape(C, P, NT, D).reshape(C, P, NT * D)
    dv = dinv[nid]
    dinv_sb = np.where(valid, dv, 1.0).astype(np.float32)
    dinv_y_sb = np.where(valid, dv, 0.0).astype(np.float32)

    # matmul RHS [128, 256] = [W_phi.T | A.T], A = W - W.T - GAMMA*I
    A = W - W.T - GAMMA * np.eye(D, dtype=np.float32)
    rhs = np.concatenate([W_phi.T, A.T], axis=1).astype(np.float32)
    bias_bcast = np.tile(bias[None, :], (P, 1)).astype(np.float32)

    in_maps = []
    for c in range(C):
        in_maps.append(
            {
                "x_in": np.ascontiguousarray(x_sb[c]),
                "dinv": np.ascontiguousarray(dinv_sb[c]),
                "dinv_y": np.ascontiguousarray(dinv_y_sb[c]),
                "src_idx": np.ascontiguousarray(src_arr[c]),
                "rhs": rhs,
                "bias_b": bias_bcast,
            }
        )
    meta = dict(
        NT=NT, NPC=NPC, D_sched=[int(v) for v in D_sched], CHT=CHT,
        node_ids=node_ids, valid=valid, N=N,
    )
    return in_maps, meta


def _postprocess(results, meta):
    NT, N = meta["NT"], meta["N"]
    node_ids, valid = meta["node_ids"], meta["valid"]
    out = np.empty((N, D), dtype=np.float32)
    for c in range(C):
        xc = results[c]["x_out"].reshape(P, NT, D)
        v = valid[c]
        out[node_ids[c][v]] = xc[v]
    return out


# ------------------------------------------------------------- device graph
def _build_graph(NT, D_sched, n_iters=NUM_ITERS, skip_collective=False):
    NPC = NT * P
    CHT = int(sum(D_sched))
    GMAX = 12  # max chunks gathered per indirect DMA (SBUF budget)

    nc = bacc.Bacc("TRN2", target_bir_lowering=False, debug=False, num_devices=C)
    x_in = nc.declare_dram_parameter("x_in", [P, NT * D], FP, isOutput=False)
    dinv_in = nc.declare_dram_parameter("dinv", [P, NT], FP, isOutput=False)
    dinv_y_in = nc.declare_dram_parameter("dinv_y", [P, NT], FP, isOutput=False)
    src_in = nc.declare_dram_parameter("src_idx", [P, CHT], I32, isOutput=False)
    rhs_in = nc.declare_dram_parameter("rhs", [P, 2 * D], FP, isOutput=False)
    bias_in = nc.declare_dram_parameter("bias_b", [P, D], FP, isOutput=False)
    x_out = nc.declare_dram_parameter("x_out", [P, NT * D], FP, isOutput=True)

    y_bounce = nc.dram_tensor("y_bounce", [NPC, D], FP)
    y_full = nc.dram_tensor("y_full", [C * NPC, D], FP, addr_space="Shared")

    off = np.zeros(NT + 1, dtype=np.int64)
    off[1:] = np.cumsum(D_sched)

    with tile.TileContext(nc) as tc:
        with (
            tc.tile_pool(name="stat", bufs=1) as stat,
            tc.tile_pool(name="sb", bufs=2) as sb,
            tc.tile_pool(name="gat", bufs=4) as gat,
            tc.tile_pool(name="ps", bufs=2, space="PSUM") as psp,
            tc.tile_pool(name="psagg", bufs=2, space="PSUM") as psagg,
        ):
            # ---- static data
            ident = stat.tile([P, P], FP)
            make_identity(nc, ident[:])
            rhs_sb = stat.tile([P, 2 * D], FP)
            nc.sync.dma_start(rhs_sb[:], rhs_in[:])
            bias_sb = stat.tile([P, D], FP)
            nc.sync.dma_start(bias_sb[:], bias_in[:])
            dinv_sb = stat.tile([P, NT], FP)
            nc.sync.dma_start(dinv_sb[:], dinv_in[:])
            dinvy_sb = stat.tile([P, NT], FP)
            nc.sync.dma_start(dinvy_sb[:], dinv_y_in[:])
            idx_sb = stat.tile([P, CHT], I32)
            nc.sync.dma_start(idx_sb[:], src_in[:])
            x_sb = stat.tile([P, NT * D], FP)
            nc.sync.dma_start(x_sb[:], x_in[:])
            y_sb = stat.tile([P, NT * D], FP)
            xa_sb = stat.tile([P, NT * D], FP)

            def phase_a(_iv):
                # ---------------- phase A: local matmuls
                for i in range(NT):
                    xt = x_sb[:, i * D:(i + 1) * D]
                    ps_t = psp.tile([P, P], FP, tag="ps_t", space="PSUM")
                    nc.tensor.transpose(out=ps_t[:], in_=xt, identity=ident[:])
                    xT = sb.tile([P, P], FP, tag="xT")
                    nc.vector.tensor_copy(out=xT[:], in_=ps_t[:])
                    ps_a = psp.tile([P, 2 * D], FP, tag="ps_a", space="PSUM")
                    nc.tensor.matmul(
                        out=ps_a[:], lhsT=xT[:], rhs=rhs_sb[:],
                        start=True, stop=True,
                    )
                    # y = xw * dinv * valid
                    nc.vector.tensor_scalar(
                        out=y_sb[:, i * D:(i + 1) * D], in0=ps_a[:, 0:D],
                        scalar1=dinvy_sb[:, i:i + 1], scalar2=None,
                        op0=mybir.AluOpType.mult,
                    )
                    # xa = x@A.T + bias
                    nc.vector.tensor_tensor(
                        out=xa_sb[:, i * D:(i + 1) * D], in0=ps_a[:, D:2 * D],
                        in1=bias_sb[:], op=mybir.AluOpType.add,
                    )
                # y rows: y_bounce[slot*NT + tile] = y_sb[slot, tile*D:...]
                # flat copy: y_sb [P, NT*D] -> y_bounce [NPC, D] row-major is
                # exactly the same bytes laid out [P][NT][D] -> row p*NT+t. ✓
                nc.sync.dma_start(
                    out=y_bounce[:].rearrange("(p t) d -> p (t d)", p=P),
                    in_=y_sb[:],
                )
                if skip_collective:
                    # timing-only variant: local copy into own shard slot
                    nc.sync.dma_start(
                        out=y_full[0:NPC, :], in_=y_bounce[:],
                    )
                else:
                    nc.gpsimd.collective_compute(
                        "AllGather",
                        mybir.AluOpType.bypass,
                        replica_groups=[list(range(C))],
                        ins=[y_bounce[:].opt()],
                        outs=[y_full[:].opt()],
                    )

            def phase_c(_iv):
                # ---------------- phase C: gather + aggregate per dst tile
                for i in range(NT):
                    Di = int(D_sched[i])
                    ps_g = psagg.tile([P, D], FP, tag="agg", space="PSUM")
                    # one [P,1]->[P,D] indirect gather per chunk (proven
                    # HW semantics), matmul-accumulate into PSUM
                    for k in range(Di):
                        yb = gat.tile([P, D], FP, tag="ybig")
                        nc.gpsimd.indirect_dma_start(
                            out=yb[:],
                            out_offset=None,
                            in_=y_full[:],
                            in_offset=bass.IndirectOffsetOnAxis(
                                ap=idx_sb[:, off[i] + k: off[i] + k + 1],
                                axis=0,
                            ),
                        )
                        nc.tensor.matmul(
                            out=ps_g[:], lhsT=ident[:], rhs=yb[:],
                            start=(k == 0), stop=False,
                        )
                    # self-loop: + y_tile
                    nc.tensor.matmul(
                        out=ps_g[:], lhsT=ident[:],
                        rhs=y_sb[:, i * D:(i + 1) * D],
                        start=False, stop=True,
                    )
                    # epilogue: t3 = agg * dinv ; t4 = t3 + xa ; h = tanh(t4)
                    t3 = sb.tile([P, D], FP, tag="t3")
                    nc.vector.tensor_scalar(
                        out=t3[:], in0=ps_g[:], scalar1=dinv_sb[:, i:i + 1],
                        scalar2=None, op0=mybir.AluOpType.mult,
                    )
                    t4 = sb.tile([P, D], FP, tag="t4")
                    nc.vector.tensor_tensor(
                        out=t4[:], in0=t3[:], in1=xa_sb[:, i * D:(i + 1) * D],
                        op=mybir.AluOpType.add,
                    )
                    h = sb.tile([P, D], FP, tag="h")
                    nc.scalar.activation(
                        out=h[:], in_=t4[:],
                        func=mybir.ActivationFunctionType.Tanh,
                    )
                    h1 = sb.tile([P, D], FP, tag="h1")
                    nc.scalar.activation(
                        out=h1[:], in_=h[:],
                        func=mybir.ActivationFunctionType.Copy, scale=EPSILON,
                    )
                    nc.vector.tensor_tensor(
                        out=x_sb[:, i * D:(i + 1) * D],
                        in0=x_sb[:, i * D:(i + 1) * D], in1=h1[:],
                        op=mybir.AluOpType.add,
                    )

            for _it in range(n_iters):
                phase_a(_it)
                # 1-trip loop: the back-edge resets the SWDGE sem lane,
                # which otherwise overflows its 16-bit wait field after
                # ~4095 indirect DMAs in straight-line code.
                with tc.For_i(0, 1, 1) as _iv:
                    phase_c(_iv)
            nc.sync.dma_start(out=x_out[:], in_=x_sb[:])
    nc.compile()
    return nc


# ------------------------------------------------------------------- driver
_LAST = {}


def kernel(x, edge_index, W, W_phi, bias):
    x = np.asarray(x, dtype=np.float32)
    edge_index = np.asarray(edge_index, dtype=np.int32)
    W = np.asarray(W, dtype=np.float32)
    W_phi = np.asarray(W_phi, dtype=np.float32)
    bias = np.asarray(bias, dtype=np.float32)

    in_maps, meta = _preprocess(x, edge_index, W, W_phi, bias)
    nc = _build_graph(meta["NT"], meta["D_sched"])
    trace = os.environ.get("BASS_PROFILE", "0") == "1"
    res = run_bass_kernel_spmd(
        nc, in_maps, core_ids=list(range(C)), trace=trace
    )
    _LAST["res"] = res
    _LAST["meta"] = meta
    return _postprocess(res.results, meta)



# revision 2
# speedup vs baseline: 9.3085x; 9.3085x over previous
"""AntiSymmetricConv (GNN message passing) on 8 TRN2 NeuronCores — v2.

Strategy (dst-sharded pull mode, bulk dma_gather + scatter-matrix matmuls):
  - Host: degree-sort dst nodes, tiles round-robin over 8 cores (identical
    static schedule). y_full rows laid out [core][slot][tile]; split into
    4 banks of 25088 rows so bank-relative indices fit dma_gather's int16.
    Per core, per dst tile, in-edges are packed (no per-slot alignment) into
    128-edge chunks grouped by source bank; chunk counts padded to the max
    over cores so the program is SPMD-identical. Per chunk: 128 int16
    bank-relative gather indices + a 128-entry slot map.
  - Device, per iteration:
      phase A per tile: xT = transpose(x) (PE); [y|xa] = xT.T @ [Wphi.T|A.T]
        (one fp32 matmul); y = y*dinv_y -> bf16; xa = xa+bias -> bf16.
      y (bf16, 3.2MB) -> DRAM bounce -> AllGather -> y_full [100352,128] bf16.
      phase C in blocks of <=12 tiles: per (block, bank) the chunk run is
        fetched by dma_gather instructions of <=15 chunks (1920 idx; SWDGE
        ring cap is 128 descriptors = 2048 idx). Per chunk: S[pos,slot] =
        (iota==slotmap) on DVE (bf16), then PE matmul-accumulates S.T @ yb
        into the tile's PSUM bank (banks consumed in gather order, so the
        gather buffer pool rotates without deadlock).
      epilogue per tile: agg=(psum+y_tile)*dinv; h=tanh(agg+xa);
        x += eps*h (fp32 state in SBUF).
  - Output: x shards to DRAM; host inverts the permutation.
"""

import os

import numpy as np
import ml_dtypes

import concourse.bacc as bacc
import concourse.bass as bass
import concourse.mybir as mybir
import concourse.tile as tile
from concourse.bass_utils import run_bass_kernel_spmd
from concourse.masks import make_identity

GAMMA = 0.1
EPSILON = 0.1
NUM_ITERS = 4
P = 128   # partitions / tile size
C = 8     # cores
D = 128   # feature dim
NBANK = 4
GCAP = 96    # max chunks per dma_gather (G=96 OK on HW w/ single_packet=False)
BT = 4       # tiles per phase-C block (PSUM: 4 agg banks + 4 phase-A banks)

FP = mybir.dt.float32
BF = mybir.dt.bfloat16
I16 = mybir.dt.int16


# ----------------------------------------------------------------- host prep
def _preprocess(x, edge_index, W, W_phi, bias):
    N = x.shape[0]
    E = edge_index.shape[1]
    src, dst = edge_index[0].astype(np.int64), edge_index[1].astype(np.int64)

    deg = np.bincount(dst, minlength=N).astype(np.float64) + 1.0
    dinv = (1.0 / np.sqrt(deg)).astype(np.float32)

    order = np.argsort(-deg, kind="stable")
    rank = np.empty(N, dtype=np.int64)
    rank[order] = np.arange(N)

    n_tiles_global = -(-N // P)
    NT = -(-n_tiles_global // C)
    NPC = NT * P
    BR = C * NPC // NBANK
    assert BR <= 32768 and C * NPC % NBANK == 0

    g = rank // P
    core_of = g % C
    tile_of = g // C
    slot_of = rank % P
    pos = core_of * NPC + slot_of * NT + tile_of  # y_full row of each node

    # per-edge fields
    er = rank[dst]
    e_core = (er // P) % C
    e_tile = (er // P) // C
    e_slot = er % P
    sp = pos[src]
    e_bank = sp // BR
    e_rel = (sp - e_bank * BR).astype(np.int16)

    # group edges by (core, tile, bank); stable order
    key = (e_core * NT + e_tile) * NBANK + e_bank
    eorder = np.argsort(key, kind="stable")
    key_s = key[eorder]
    rel_s = e_rel[eorder]
    slot_s = e_slot[eorder]
    within = np.arange(E) - np.searchsorted(key_s, key_s)

    cnt = np.bincount(key, minlength=C * NT * NBANK).reshape(C, NT, NBANK)
    n_chunks_ib = (-(-cnt // P)).max(axis=0)  # [NT, NBANK] shared schedule

    # chunk_start in (block, bank-major) global order
    chunk_start = np.zeros((NT, NBANK), dtype=np.int64)
    blocks = []
    running = 0
    for i0 in range(0, NT, BT):
        tiles = list(range(i0, min(i0 + BT, NT)))
        gathers = []
        for b in range(NBANK):
            run0 = running
            for i in tiles:
                chunk_start[i, b] = running
                running += int(n_chunks_ib[i, b])
            run_n = running - run0
            # split run into <=GCAP gathers
            s = run0
            while s < running:
                n = min(GCAP, running - s)
                gathers.append((b, s, n))
                s += n
        blocks.append({"tiles": tiles, "gathers": gathers})
    CHT = running

    # per-tile chunk list (global chunk id, in bank-major consumption order)
    tile_chunks = []
    for i in range(NT):
        lst = []
        for b in range(NBANK):
            for k in range(int(n_chunks_ib[i, b])):
                lst.append(int(chunk_start[i, b]) + k)
        tile_chunks.append(lst)

    # annotate blocks: map global chunk id -> (gather idx within block, offset)
    for blk in blocks:
        cmap = {}
        for gi, (b, s, n) in enumerate(blk["gathers"]):
            for k in range(n):
                cmap[s + k] = (gi, k)
        blk["cmap"] = cmap

    # fill per-core idx / slot arrays
    # flat chunk-position arrays [C, CHT*128]
    cp_idx = np.zeros((C, CHT * P), dtype=np.int16)
    cp_slot = np.full((C, CHT * P), 255, dtype=np.float32)
    ec = key_s // (NT * NBANK)
    eb = key_s % NBANK
    ei = (key_s // NBANK) % NT
    gchunk = chunk_start[ei, eb] + within // P
    cpos = gchunk * P + within % P
    cp_idx[ec, cpos] = rel_s
    cp_slot[ec, cpos] = slot_s.astype(np.float32)

    # idx16 [C, 128, CHT*8]: index g of chunk j at [16k + g%16, j*8 + g//16]
    idx16 = cp_idx.reshape(C, CHT * 8, 16).transpose(0, 2, 1)  # [C, 16, CHT*8]
    idx16 = np.tile(idx16, (1, 8, 1))                          # replicate to 128
    # slotm [C, 128, CHT]: partition = chunk position, col = chunk
    slotm = cp_slot.reshape(C, CHT, P).transpose(0, 2, 1).astype(np.float32)

    # per-core node data
    node_ids = np.full((C, P, NT), -1, dtype=np.int64)
    node_ids[core_of, slot_of, tile_of] = np.arange(N)
    valid = node_ids >= 0
    nid = np.where(valid, node_ids, 0)
    x_gather = x[nid.reshape(C, -1)]
    x_gather[~valid.reshape(C, -1)] = 0.0
    x_sb = x_gather.reshape(C, P, NT, D).reshape(C, P, NT * D)
    dv = dinv[nid]
    dinv_sb = np.where(valid, dv, 1.0).astype(np.float32)
    dinv_y_sb = np.where(valid, dv, 0.0).astype(np.float32)

    A = W - W.T - GAMMA * np.eye(D, dtype=np.float32)
    rhs = np.concatenate([W_phi.T, A.T], axis=1).astype(np.float32)
    bias_bcast = np.tile(bias[None, :], (P, 1)).astype(np.float32)

    in_maps = []
    for c in range(C):
        in_maps.append(
            {
                "x_in": np.ascontiguousarray(x_sb[c]),
                "dinv": np.ascontiguousarray(dinv_sb[c]),
                "dinv_y": np.ascontiguousarray(dinv_y_sb[c]),
                "idx16": np.ascontiguousarray(idx16[c]),
                "slotm": np.ascontiguousarray(slotm[c]),
                "rhs": rhs,
                "bias_b": bias_bcast,
            }
        )
    sched = dict(
        NT=NT, NPC=NPC, BR=BR, CHT=CHT, blocks=blocks,
        tile_chunks=tile_chunks,
    )
    meta = dict(node_ids=node_ids, valid=valid, N=N, sched=sched)
    return in_maps, meta


def _postprocess(results, meta):
    node_ids, valid, N = meta["node_ids"], meta["valid"], meta["N"]
    NT = meta["sched"]["NT"]
    out = np.empty((N, D), dtype=np.float32)
    for c in range(C):
        xc = results[c]["x_out"].reshape(P, NT, D)
        v = valid[c]
        out[node_ids[c][v]] = xc[v]
    return out


# ------------------------------------------------------------- device graph
def _build_graph(sched, n_iters=NUM_ITERS):
    NT = sched["NT"]
    NPC = sched["NPC"]
    BR = sched["BR"]
    CHT = sched["CHT"]
    blocks = sched["blocks"]
    tile_chunks = sched["tile_chunks"]

    nc = bacc.Bacc("TRN2", target_bir_lowering=False, debug=False, num_devices=C)
    x_in = nc.declare_dram_parameter("x_in", [P, NT * D], FP, isOutput=False)
    dinv_in = nc.declare_dram_parameter("dinv", [P, NT], FP, isOutput=False)
    dinv_y_in = nc.declare_dram_parameter("dinv_y", [P, NT], FP, isOutput=False)
    idx_in = nc.declare_dram_parameter("idx16", [P, CHT * 8], I16, isOutput=False)
    slot_in = nc.declare_dram_parameter("slotm", [P, CHT], FP, isOutput=False)
    rhs_in = nc.declare_dram_parameter("rhs", [P, 2 * D], FP, isOutput=False)
    bias_in = nc.declare_dram_parameter("bias_b", [P, D], FP, isOutput=False)
    x_out = nc.declare_dram_parameter("x_out", [P, NT * D], FP, isOutput=True)

    y_bounce = nc.dram_tensor("y_bounce", [NPC, D], BF)
    y_full = nc.dram_tensor("y_full", [C * NPC, D], BF, addr_space="Shared")

    with tile.TileContext(nc) as tc:
        with (
            tc.tile_pool(name="stat", bufs=1) as stat,
            tc.tile_pool(name="sb", bufs=3) as sbp,
            tc.tile_pool(name="gat", bufs=2) as gat,
            tc.tile_pool(name="spool", bufs=4) as spool,
            tc.tile_pool(name="ps", bufs=2, space="PSUM") as psp,
            tc.tile_pool(name="psagg", bufs=1, space="PSUM") as psagg,
        ):
            # ---- static data
            ident = stat.tile([P, P], FP)
            make_identity(nc, ident[:])
            iota_row = stat.tile([P, P], FP)
            nc.gpsimd.iota(
                iota_row[:], pattern=[[1, P]], base=0, channel_multiplier=0,
                allow_small_or_imprecise_dtypes=True,
            )
            rhs_sb = stat.tile([P, 2 * D], FP)
            nc.sync.dma_start(rhs_sb[:], rhs_in[:])
            bias_sb = stat.tile([P, D], FP)
            nc.sync.dma_start(bias_sb[:], bias_in[:])
            dinv_sb = stat.tile([P, NT], FP)
            nc.sync.dma_start(dinv_sb[:], dinv_in[:])
            dinvy_sb = stat.tile([P, NT], FP)
            nc.sync.dma_start(dinvy_sb[:], dinv_y_in[:])
            idx_sb = stat.tile([P, CHT * 8], I16)
            nc.sync.dma_start(idx_sb[:], idx_in[:])
            slot_sb = stat.tile([P, CHT], FP)
            nc.sync.dma_start(slot_sb[:], slot_in[:])
            x_sb = stat.tile([P, NT * D], FP)
            nc.sync.dma_start(x_sb[:], x_in[:])
            y_sb = stat.tile([P, NT * D], BF)
            xa_sb = stat.tile([P, NT * D], BF)

            def phase_a():
                for i in range(NT):
                    xt = x_sb[:, i * D:(i + 1) * D]
                    ps_t = psp.tile([P, P], FP, tag="ps_t", space="PSUM")
                    nc.tensor.transpose(out=ps_t[:], in_=xt, identity=ident[:])
                    xT = sbp.tile([P, P], FP, tag="xT")
                    nc.vector.tensor_copy(out=xT[:], in_=ps_t[:])
                    ps_a = psp.tile([P, 2 * D], FP, tag="ps_a", space="PSUM")
                    nc.tensor.matmul(
                        out=ps_a[:], lhsT=xT[:], rhs=rhs_sb[:],
                        start=True, stop=True,
                    )
                    nc.vector.tensor_scalar(
                        out=y_sb[:, i * D:(i + 1) * D], in0=ps_a[:, 0:D],
                        scalar1=dinvy_sb[:, i:i + 1], scalar2=None,
                        op0=mybir.AluOpType.mult,
                    )
                    nc.vector.tensor_tensor(
                        out=xa_sb[:, i * D:(i + 1) * D], in0=ps_a[:, D:2 * D],
                        in1=bias_sb[:], op=mybir.AluOpType.add,
                    )
                nc.sync.dma_start(
                    out=y_bounce[:].rearrange("(p t) d -> p (t d)", p=P),
                    in_=y_sb[:],
                )
                nc.gpsimd.collective_compute(
                    "AllGather",
                    mybir.AluOpType.bypass,
                    replica_groups=[list(range(C))],
                    ins=[y_bounce[:].opt()],
                    outs=[y_full[:].opt()],
                )

            def phase_c():
                for blk in blocks:
                    bufs = []
                    for (b, s, n) in blk["gathers"]:
                        buf = gat.tile([P, GCAP * D], BF, tag="gat")
                        nc.gpsimd.dma_gather(
                            buf[:, 0:n * D].rearrange("p (g d) -> p g d", d=D),
                            y_full[b * BR:(b + 1) * BR, :],
                            idx_sb[:, s * 8:(s + n) * 8],
                            n * P,
                            n * P,
                            D,
                            single_packet=False,
                        )
                        bufs.append(buf)
                    # per tile: matmul-accumulate chunks (bank-major order).
                    # one PSUM bank per tile in the block (accumulation
                    # groups are bank-granular).
                    pstiles = {}
                    for idx, i in enumerate(blk["tiles"]):
                        chunks = tile_chunks[i]
                        if not chunks:
                            continue
                        pstiles[i] = psagg.tile(
                            [P, D], FP, tag=f"agg{idx}", space="PSUM",
                            name=f"agg{idx}",
                        )[:]
                    emitted = {i: 0 for i in blk["tiles"]}
                    for gi, (b, s, n) in enumerate(blk["gathers"]):
                        for i in blk["tiles"]:
                            chunks = tile_chunks[i]
                            for j in chunks:
                                if not (s <= j < s + n):
                                    continue
                                off = j - s
                                S = spool.tile([P, P], BF, tag="S")
                                nc.vector.tensor_scalar(
                                    out=S[:], in0=iota_row[:],
                                    scalar1=slot_sb[:, j:j + 1], scalar2=None,
                                    op0=mybir.AluOpType.is_equal,
                                )
                                k = emitted[i]
                                nc.tensor.matmul(
                                    out=pstiles[i],
                                    lhsT=S[:],
                                    rhs=bufs[gi][:, off * D:(off + 1) * D],
                                    start=(k == 0),
                                    stop=(k == len(chunks) - 1),
                                )
                                emitted[i] = k + 1
                    # epilogue per tile
                    for i in blk["tiles"]:
                        chunks = tile_chunks[i]
                        ysl = y_sb[:, i * D:(i + 1) * D]
                        t0 = sbp.tile([P, D], FP, tag="t0")
                        if chunks:
                            nc.vector.tensor_tensor(
                                out=t0[:], in0=pstiles[i], in1=ysl,
                                op=mybir.AluOpType.add,
                            )
                        else:
                            nc.vector.tensor_copy(out=t0[:], in_=ysl)
                        t3 = sbp.tile([P, D], FP, tag="t3")
                        nc.vector.tensor_scalar(
                            out=t3[:], in0=t0[:],
                            scalar1=dinv_sb[:, i:i + 1], scalar2=None,
                            op0=mybir.AluOpType.mult,
                        )
                        t4 = sbp.tile([P, D], FP, tag="t4")
                        nc.vector.tensor_tensor(
                            out=t4[:], in0=t3[:],
                            in1=xa_sb[:, i * D:(i + 1) * D],
                            op=mybir.AluOpType.add,
                        )
                        h = sbp.tile([P, D], FP, tag="h")
                        nc.scalar.activation(
                            out=h[:], in_=t4[:],
                            func=mybir.ActivationFunctionType.Tanh,
                        )
                        h1 = sbp.tile([P, D], FP, tag="h1")
                        nc.scalar.activation(
                            out=h1[:], in_=h[:],
                            func=mybir.ActivationFunctionType.Copy,
                            scale=EPSILON,
                        )
                        nc.vector.tensor_tensor(
                            out=x_sb[:, i * D:(i + 1) * D],
                            in0=x_sb[:, i * D:(i + 1) * D], in1=h1[:],
                            op=mybir.AluOpType.add,
                        )

            # NOTE: do NOT wrap iterations in tc.For_i — InstCollectiveCompute
            # does not re-execute inside hardware loops here (verified: output
            # matches a stale-y reference), silently computing the wrong thing.
            for _it in range(n_iters):
                phase_a()
                phase_c()
            nc.sync.dma_start(out=x_out[:], in_=x_sb[:])
    nc.compile()
    return nc


# ------------------------------------------------------------------- driver
_LAST = {}


def kernel(x, edge_index, W, W_phi, bias):
    x = np.asarray(x, dtype=np.float32)
    edge_index = np.asarray(edge_index, dtype=np.int32)
    W = np.asarray(W, dtype=np.float32)
    W_phi = np.asarray(W_phi, dtype=np.float32)
    bias = np.asarray(bias, dtype=np.float32)

    in_maps, meta = _preprocess(x, edge_index, W, W_phi, bias)
    nc = _build_graph(meta["sched"])
    res = run_bass_kernel_spmd(nc, in_maps, core_ids=list(range(C)))
    _LAST["res"] = res
    _LAST["meta"] = meta
    return _postprocess(res.results, meta)
